# revision 1
# baseline (speedup 1.0000x reference)
"""Distributed Trainium2 Bass kernel for nn_NodeFeat (2-hop Chebyshev-style GNN
feature expansion + edge gather), 8 NeuronCores.

Node sharding per the problem's sharding hint:
  - 50000 nodes padded to 50176 = 8 x 6272; core c owns rows [6272c, 6272c+6272).
  - adjacency rows are pre-sorted; each core handles the edges whose ROW is in
    its shard, packed per 128-row tile into NCHUNK=18 chunks of 128 slots
    (dummy slots use an out-of-bounds index -> DMA descriptor skipped).
  - hop1: indirect-DMA gather of x[col] rows, scaled per-edge by
    {1, rsqrt(deg_col), sqrt(deg_col)} into a [128,192] fp16 moving operand;
    segment-sum on TensorE via a one-hot selector (is_equal of rowloc vs iota)
    accumulated in PSUM; ScalarE evacuates with the 1/deg row scale.
  - one on-chip AllGather of the per-core y1 shard between hops.
  - hop2: same machinery gathering y1 rows, then minus xs0.
  - final: edge endpoints partitioned by owner core (host all-to-all
    bookkeeping); each core gathers its [xs0|y1|xs2] rows, transposes [9,64]
    -> [64,9] on-chip, writes packed rows; host scatters into [2,32768,64,9].

All floating-point math runs on device; the host only shards, pads, reorders
and reassembles (index bookkeeping).
"""
import numpy as np

import concourse.bass as bass
import concourse.mybir as mybir
import concourse.tile as tile
from concourse.bass_utils import run_bass_kernel_spmd

# ---------------- hardcoded problem geometry ----------------
N = 50000
D = 64
EQ = 32768
P = 128
NC = 8                   # cores
NT = 49                  # row tiles per core
NSH = NT * P             # 6272 rows per core
NPAD = NSH * NC          # 50176
NCHUNK = 18              # 128-edge chunks per row tile
FCH = 66                 # final-gather chunks per core (66*128 = 8448 slots)
PC = 6                   # final-gather chunks per piece (11 pieces)
BIG = 10 ** 7            # out-of-bounds index -> DMA descriptor skipped
F32 = mybir.dt.float32
F16 = mybir.dt.float16
I32 = mybir.dt.int32
EDGE_COLS = NT * NCHUNK  # 882

_prog_cache = {}


class _TC(tile.TileContext):
    """TileContext whose final drain splits sem waits one-per-instruction
    (this walrus rejects >1 sync wait on an instruction)."""

    def _drain_and_barrier(self, tick_clock, wait_clock):
        nc = self.nc
        probe = nc.sync.nop()
        wait_clock.add_sem_waits(
            probe.ins, tile.ScopedClock({None: tick_clock.global_clock}))
        si = probe.ins.sync_info
        waits = list(si.on_wait) if si and si.on_wait else []
        if si is not None:
            si.on_wait = waits[:1]
        for w in waits[1:]:
            n2 = nc.sync.nop()
            if n2.ins.sync_info is None:
                n2.ins.sync_info = mybir.SyncInfo(on_wait=[w], on_update=[])
            else:
                n2.ins.sync_info.on_wait = [w]
        nc.sync.drain()
        nc.all_engine_barrier()
        popped = nc._tile_sem_poison_stack.pop()
        assert popped is self._sem_poison
        nc.clear_and_free_semaphores(list(self.sems.allocated().values()))
        nc.all_engine_barrier()


def _split_multi_waits(nc):
    for fn in nc.m.functions:
        for blk in fn.blocks:
            new_list = []
            for inst in blk.instructions:
                si = inst.sync_info
                waits = list(si.on_wait) if si and si.on_wait else []
                if len(waits) > 1:
                    for j, w in enumerate(waits[:-1]):
                        nop = mybir.InstNoOp(
                            name=f"{inst.name}-ws{j}",
                            engine=inst.engine,
                            ins=[], outs=[],
                            sync_info=mybir.SyncInfo(on_wait=[w], on_update=[]),
                        )
                        nc.register_instruction(nop, overwrite=True)
                        new_list.append(nop)
                    si.on_wait = waits[-1:]
                new_list.append(inst)
            blk.instructions[:] = new_list


def _dims(ap, dims):
    """Same tensor+offset as `ap`, explicit [stride(elem), nelem] dims."""
    return bass.AP(ap.tensor, ap.offset, dims)


def _build_program(ablate=()):
    """ablate: subset of {"hop1","gather1","ag","hop2","gather2","final","gatherf"}
    to SKIP (for performance ablation only — results become wrong)."""
    ab = set(ablate)
    nc = bass.Bass("TRN2", target_bir_lowering=False, debug=False, num_devices=NC)

    x_full = nc.dram_tensor("x_full", [NPAD, D], F32, kind="ExternalInput")
    x_sh = nc.dram_tensor("x_sh", [NSH, D], F32, kind="ExternalInput")
    degsh_in = nc.dram_tensor("degsh", [P, NT], F32, kind="ExternalInput")
    idx1_in = nc.dram_tensor("idx1", [P, EDGE_COLS], I32, kind="ExternalInput")
    rowloc_in = nc.dram_tensor("rowloc", [P, EDGE_COLS], F16, kind="ExternalInput")
    degcol_in = nc.dram_tensor("degcol", [P, EDGE_COLS], F32, kind="ExternalInput")
    fidx_loc_in = nc.dram_tensor("fidx_loc", [P, FCH], I32, kind="ExternalInput")
    fidx_mid_in = nc.dram_tensor("fidx_mid", [P, FCH], I32, kind="ExternalInput")
    iota_in = nc.dram_tensor("iota", [P, P], F16, kind="ExternalInput")

    out_f = nc.dram_tensor("out_f", [FCH * P, 576], F32, kind="ExternalOutput")

    y1_bounce = nc.dram_tensor("y1_bounce", [NSH, 192], F32)
    y1full = nc.dram_tensor("y1full", [NPAD, 192], F32, addr_space="Shared")
    xs0_l = nc.dram_tensor("xs0_l", [NSH, 192], F32)
    xs2_l = nc.dram_tensor("xs2_l", [NSH, 192], F32)

    eq = mybir.AluOpType.is_equal
    mult = mybir.AluOpType.mult
    sub = mybir.AluOpType.subtract
    COPY = mybir.ActivationFunctionType.Copy
    SQRT = mybir.ActivationFunctionType.Sqrt

    with _TC(nc) as tc, nc.allow_low_precision(reason="fp16 matmul operands; PSUM accumulates in f32"), \
            nc.gpsimd.register("bnd_pad") as bnd_pad, \
            nc.gpsimd.register("bnd_sh") as bnd_sh:
        nc.gpsimd.reg_mov(bnd_pad, NPAD - 1)
        nc.gpsimd.reg_mov(bnd_sh, NSH - 1)
        with (
            tc.tile_pool(name="const", bufs=1) as cp,
            tc.tile_pool(name="v1", bufs=3) as v1p,
            tc.tile_pool(name="s", bufs=3) as sp_,
            tc.tile_pool(name="v3", bufs=3) as v3p,
            tc.tile_pool(name="rq", bufs=3) as rqp,
            tc.tile_pool(name="ev", bufs=3) as evp,
            tc.tile_pool(name="x0", bufs=3) as x0p,
            tc.tile_pool(name="v2", bufs=3) as v2p,
            tc.tile_pool(name="g", bufs=2) as gp,
            tc.tile_pool(name="st", bufs=2) as stp,
            tc.tile_pool(name="psum", bufs=4, space="PSUM") as pp,
        ):
            iota_t = cp.tile([P, P], F16)
            nc.sync.dma_start(out=iota_t[:], in_=iota_in[:])
            idx1_t = cp.tile([P, EDGE_COLS], I32)
            nc.sync.dma_start(out=idx1_t[:], in_=idx1_in[:])
            rowloc_t = cp.tile([P, EDGE_COLS], F16)
            nc.sync.dma_start(out=rowloc_t[:], in_=rowloc_in[:])
            degcol_t = cp.tile([P, EDGE_COLS], F32)
            nc.sync.dma_start(out=degcol_t[:], in_=degcol_in[:])
            degsh_t = cp.tile([P, NT], F32)
            nc.sync.dma_start(out=degsh_t[:], in_=degsh_in[:])
            fidx_loc_t = cp.tile([P, FCH], I32)
            nc.sync.dma_start(out=fidx_loc_t[:], in_=fidx_loc_in[:])
            fidx_mid_t = cp.tile([P, FCH], I32)
            nc.sync.dma_start(out=fidx_mid_t[:], in_=fidx_mid_in[:])

            def build_s(t):
                s_t = sp_.tile([P, NCHUNK, P], F16, tag="s")
                rl = rowloc_t[:, t * NCHUNK:(t + 1) * NCHUNK]
                rl_b = rl.to_broadcast([P, NCHUNK, P])
                io = iota_t[:]
                io_b = _dims(io, [io.ap[0], [0, NCHUNK], io.ap[1]])
                nc.vector.tensor_tensor(out=s_t[:], in0=rl_b, in1=io_b, op=eq)
                return s_t

            # whole-shard precomputes (hoisted out of the tile loops)
            # rq_all[:, 0, :] = rsqrt(deg_col) f16, rq_all[:, 1, :] = sqrt f16
            rq_all = cp.tile([P, 2, EDGE_COLS], F16)
            q32_all = cp.tile([P, EDGE_COLS], F32)
            nc.scalar.activation(q32_all[:], degcol_t[:], SQRT)
            nc.vector.tensor_copy(out=rq_all[:, 1, :], in_=q32_all[:])
            nc.vector.reciprocal(rq_all[:, 0, :], q32_all[:])
            # degrev_all [P, NT] f32; rq0_all [P, 2, NT] f32 (row scales)
            degrev_all = cp.tile([P, NT], F32)
            nc.vector.reciprocal(degrev_all[:], degsh_t[:])
            rq0_all = cp.tile([P, 2, NT], F32)
            nc.scalar.activation(rq0_all[:, 1, :], degsh_t[:], SQRT)
            nc.vector.reciprocal(rq0_all[:, 0, :], rq0_all[:, 1, :])
            # xs0 block 0 = x (DRAM->DRAM strided copy, once)
            x0dst = _dims(xs0_l[:, 0:D], [[192, NSH], [1, D]])
            nc.sync.dma_start(out=x0dst, in_=x_sh[:])

            # ================= hop 1 =================
            for t in range(NT if "hop1" not in ab else 0):
                v_t = v1p.tile([P, NCHUNK, D], F32, tag="v1")
                if t < 3 or "gather1" in ab:
                    nc.gpsimd.memset(v_t[:], 0.0)
                for j in range(NCHUNK if "gather1" not in ab else 0):
                    col = t * NCHUNK + j
                    nc.gpsimd.indirect_dma_start(
                        out=v_t[:, j, :], out_offset=None, in_=x_full[:],
                        in_offset=bass.IndirectOffsetOnAxis(
                            ap=idx1_t[:, col:col + 1], axis=0),
                        bounds_check=bnd_pad, oob_is_err=False,
                    )
                s_t = build_s(t)
                rq = rq_all[:, :, t * NCHUNK:(t + 1) * NCHUNK]
                # v3 [P, NCHUNK, 192] fp16 = [x | x*r | x*q] per chunk
                v3 = v3p.tile([P, NCHUNK, 192], F16, tag="v3")
                b0 = v3[:, :, 0:D]
                nc.scalar.activation(b0, v_t[:], COPY)
                b12 = _dims(v3[:, :, D:3 * D],
                            [v3[:].ap[0], [192, NCHUNK], [D, 2], [1, D]])
                v16b = _dims(v3[:, :, 0:D],
                             [v3[:].ap[0], [192, NCHUNK], [0, 2], [1, D]])
                rqb = _dims(rq, [rq_all[:].ap[0], [1, NCHUNK],
                                 [EDGE_COLS, 2], [0, D]])
                nc.vector.tensor_tensor(out=b12, in0=v16b, in1=rqb, op=mult)
                ps = pp.tile([P, 192], F32, space="PSUM", tag="ps")
                for j in range(NCHUNK):
                    nc.tensor.matmul(
                        out=ps[:], lhsT=s_t[:, j, :], rhs=v3[:, j, :],
                        start=(j == 0), stop=(j == NCHUNK - 1))
                y1_t = evp.tile([P, 192], F32, tag="y1")
                nc.scalar.activation(y1_t[:], ps[:], COPY,
                                     scale=degrev_all[:, t:t + 1])
                nc.sync.dma_start(out=y1_bounce[t * P:(t + 1) * P, :], in_=y1_t[:])
                # xs0 blocks 1-2 = x * {rsqrt(deg_row), sqrt(deg_row)}
                x_t = x0p.tile([P, D], F32, tag="xt")
                nc.sync.dma_start(out=x_t[:], in_=x_sh[t * P:(t + 1) * P, :])
                xs0_t = x0p.tile([P, 2, D], F32, tag="xs0")
                xb = _dims(x_t[:], [x_t[:].ap[0], [0, 2], [1, D]])
                rq0b = _dims(rq0_all[:, :, t:t + 1],
                             [rq0_all[:].ap[0], [NT, 2], [0, D]])
                nc.vector.tensor_tensor(out=xs0_t[:], in0=xb, in1=rq0b, op=mult)
                x12dst = _dims(xs0_l[t * P:(t + 1) * P, D:3 * D],
                               [[192, P], [1, 2 * D]])
                nc.sync.dma_start(out=x12dst, in_=xs0_t[:])

            # ================= AllGather =================
            if "ag" not in ab:
                nc.gpsimd.collective_compute(
                "AllGather", mybir.AluOpType.bypass,
                    replica_groups=[list(range(NC))],
                    ins=[y1_bounce[:]], outs=[y1full[:]],
                )

            # ================= hop 2 =================
            for t in range(NT if "hop2" not in ab else 0):
                v2 = v2p.tile([P, NCHUNK, 192], F32, tag="v2")
                if t < 3 or "gather2" in ab:
                    nc.gpsimd.memset(v2[:], 0.0)
                for j in range(NCHUNK if "gather2" not in ab else 0):
                    col = t * NCHUNK + j
                    nc.gpsimd.indirect_dma_start(
                        out=v2[:, j, :], out_offset=None, in_=y1full[:],
                        in_offset=bass.IndirectOffsetOnAxis(
                            ap=idx1_t[:, col:col + 1], axis=0),
                        bounds_check=bnd_pad, oob_is_err=False,
                    )
                s_t = build_s(t)
                v216 = v3p.tile([P, NCHUNK, 192], F16, tag="v216")
                nc.scalar.activation(v216[:], v2[:], COPY)
                ps = pp.tile([P, 192], F32, space="PSUM", tag="ps")
                for j in range(NCHUNK):
                    nc.tensor.matmul(
                        out=ps[:], lhsT=s_t[:, j, :], rhs=v216[:, j, :],
                        start=(j == 0), stop=(j == NCHUNK - 1))
                tmp = evp.tile([P, 192], F32, tag="tmp2")
                nc.scalar.activation(tmp[:], ps[:], COPY,
                                     scale=degrev_all[:, t:t + 1])
                xs0_t = x0p.tile([P, 192], F32, tag="xs0r")
                nc.sync.dma_start(out=xs0_t[:], in_=xs0_l[t * P:(t + 1) * P, :])
                xs2_t = evp.tile([P, 192], F32, tag="xs2")
                nc.vector.tensor_tensor(out=xs2_t[:], in0=tmp[:], in1=xs0_t[:], op=sub)
                nc.sync.dma_start(out=xs2_l[t * P:(t + 1) * P, :], in_=xs2_t[:])

            # ================= final gather + transpose =================
            tables = [xs0_l, y1full, xs2_l]
            fidx = [fidx_loc_t, fidx_mid_t, fidx_loc_t]
            bounds = [bnd_sh, bnd_pad, bnd_sh]
            for pc_i in range(FCH // PC if "final" not in ab else 0):
                gs = []
                for h in range(3):
                    g = gp.tile([P, PC, 192], F32, tag=f"g{h}")
                    if pc_i < 2 or "gatherf" in ab:
                        nc.gpsimd.memset(g[:], 0.0)
                    for j in range(PC if "gatherf" not in ab else 0):
                        col = pc_i * PC + j
                        nc.gpsimd.indirect_dma_start(
                            out=g[:, j, :], out_offset=None, in_=tables[h][:],
                            in_offset=bass.IndirectOffsetOnAxis(
                                ap=fidx[h][:, col:col + 1], axis=0),
                            bounds_check=bounds[h], oob_is_err=False,
                        )
                    gs.append(g)
                stage = stp.tile([P, PC, D * 9], F32, tag="stage")
                for k in range(9):
                    h, b = divmod(k, 3)
                    src = gs[h][:, :, b * D:(b + 1) * D]
                    dst = _dims(stage[:, :, k:k + 1],
                                [stage[:].ap[0], [D * 9, PC], [9, D]])
                    if k % 2 == 0:
                        nc.vector.tensor_copy(out=dst, in_=src)
                    else:
                        nc.scalar.activation(dst, src, COPY)
                obase = out_f[pc_i * PC * P:(pc_i + 1) * PC * P, :]
                orows = _dims(obase, [[576, P], [P * 576, PC], [1, 576]])
                nc.sync.dma_start(out=orows, in_=stage[:])

    _split_multi_waits(nc)
    return nc


def _plan(x, deg, adj_row, adj_col, edge):
    """Host-side sharding: pure index bookkeeping + input reordering."""
    x = np.asarray(x, np.float32)
    deg = np.asarray(deg, np.float32).reshape(-1)
    adj_row = np.asarray(adj_row, np.int64)
    adj_col = np.asarray(adj_col, np.int64)
    edge = np.asarray(edge, np.int64)

    x_full = np.zeros((NPAD, D), np.float32)
    x_full[:N] = x
    iota_np = np.tile(np.arange(P, dtype=np.float16), (P, 1))
    ep = edge.reshape(-1)

    in_maps, positions = [], []
    for c in range(NC):
        r0 = c * NSH
        idx1 = np.full((P, EDGE_COLS), BIG, np.int32)
        rowloc = np.full((P, EDGE_COLS), -1.0, np.float16)
        degcol = np.ones((P, EDGE_COLS), np.float32)
        for t in range(NT):
            base = r0 + t * P
            lo = np.searchsorted(adj_row, base, side="left")
            hi = np.searchsorted(adj_row, base + P, side="left")
            n_e = hi - lo
            assert n_e <= NCHUNK * P, f"tile overflow: {n_e}"
            sl = np.arange(n_e)
            jj, pp_ = divmod(sl, P)
            colbase = t * NCHUNK
            idx1[pp_, colbase + jj] = adj_col[lo:hi]
            rowloc[pp_, colbase + jj] = (adj_row[lo:hi] - base).astype(np.float16)
            degcol[pp_, colbase + jj] = deg[adj_col[lo:hi]]
        real = min(NSH, max(0, N - r0))
        dlocal = np.ones(NSH, np.float32)
        dlocal[:real] = deg[r0:r0 + real]
        degsh = dlocal.reshape(NT, P).T.copy()

        x_shard = np.zeros((NSH, D), np.float32)
        x_shard[:real] = x[r0:r0 + real]

        mine = np.nonzero((ep >= r0) & (ep < r0 + NSH))[0]
        n_c = len(mine)
        assert n_c <= FCH * P, f"endpoint overflow: {n_c}"
        fidx_loc = np.full((P, FCH), BIG, np.int32)
        fidx_mid = np.full((P, FCH), BIG, np.int32)
        sl = np.arange(n_c)
        jj, pp_ = divmod(sl, P)
        fidx_loc[pp_, jj] = (ep[mine] - r0).astype(np.int32)
        fidx_mid[pp_, jj] = ep[mine].astype(np.int32)
        positions.append(mine)

        in_maps.append({
            "x_full": x_full,
            "x_sh": x_shard,
            "degsh": degsh,
            "idx1": idx1,
            "rowloc": rowloc,
            "degcol": degcol,
            "fidx_loc": fidx_loc,
            "fidx_mid": fidx_mid,
            "iota": iota_np,
        })
    return in_maps, positions


def _assemble(results, positions):
    out = np.zeros((2 * EQ, 576), np.float32)
    for c in range(NC):
        rows = results[c]["out_f"]
        n_c = len(positions[c])
        out[positions[c]] = rows[:n_c]
    return out.reshape(2, EQ, D, 9)


def kernel(x, deg, adj_row, adj_col, edge):
    import time
    if "nc" not in _prog_cache:
        t0 = time.time()
        _prog_cache["nc"] = _build_program()
        print(f"[kernel] program build: {time.time()-t0:.1f}s", flush=True)
    nc = _prog_cache["nc"]
    t0 = time.time()
    in_maps, positions = _plan(x, deg, adj_row, adj_col, edge)
    print(f"[kernel] host plan: {time.time()-t0:.1f}s", flush=True)
    t0 = time.time()
    res = run_bass_kernel_spmd(nc, in_maps, list(range(NC)))
    print(f"[kernel] compile+run: {time.time()-t0:.1f}s", flush=True)
    return _assemble(res.results, positions)



# revision 2
# speedup vs baseline: 1.1348x; 1.1348x over previous
"""Distributed Trainium2 Bass kernel for nn_NodeFeat (2-hop Chebyshev-style GNN
feature expansion + edge gather), 8 NeuronCores — dma_gather version.

vs the v1 baseline (per-chunk gpsimd indirect DMA):
  - all gathers use the vectorized SWDGE dma_gather extended instruction
    (library 'mlp'), int16 indices replicated across the 8 Q7 cores'
    16-partition groups; tables > 32768 rows are gathered in two halves
    (base-offset split) with chunks pre-partitioned host-side by col < 32768.
  - x is uploaded as the per-core shard only and AllGather'ed on device.
  - y1 is stored/AllGather'ed as fp16 padded to 256 elems (512B rows) and
    gathered directly as fp16 matmul input (no f32 gather + convert).
  - final stage gathers ONE fused node table [xs0|y1|y2] (640 f16 = 1280B
    rows) instead of 3 f32 tables, and writes fp16 output rows.
  - adaptive program structure (chunk counts per row-tile, max over cores)
    computed from the input at runtime; program cached per structure.
"""
import numpy as np

import concourse.bass as bass
import concourse.mybir as mybir
import concourse.tile as tile
from concourse import library_config
from concourse.bass_utils import run_bass_kernel_spmd
from concourse.library_overlay import lower_extended_insts

# ---------------- hardcoded problem geometry ----------------
N = 50000
D = 64
EQ = 32768
P = 128
NC = 8                   # cores
NT = 49                  # row tiles per core
NSH = NT * P             # 6272 rows per core
NPAD = NSH * NC          # 50176
SPLIT = 32768            # int16 gather index limit
HIROWS = NPAD - SPLIT    # 17408
CAP = 28                 # max chunks per gather piece (SWDGE ring: 256 descs
                         # per engine; num_idxs/16+1 must stay below that)
FPC = 6                  # final-gather chunks per piece
GMAX = 7                 # max chunks per single dma_gather instruction
FT = 640                 # fused final-table row elems (f16; 1280B, %256==0)
F32 = mybir.dt.float32
F16 = mybir.dt.float16
F8 = mybir.dt.float8e4
I32 = mybir.dt.int32
I16 = mybir.dt.int16

_prog_cache = {}


class _TC(tile.TileContext):
    """TileContext whose final drain splits sem waits one-per-instruction
    (this walrus rejects >1 sync wait on an instruction)."""

    def _drain_and_barrier(self, tick_clock, wait_clock):
        nc = self.nc
        probe = nc.sync.nop()
        wait_clock.add_sem_waits(
            probe.ins, tile.ScopedClock({None: tick_clock.global_clock}))
        si = probe.ins.sync_info
        waits = list(si.on_wait) if si and si.on_wait else []
        if si is not None:
            si.on_wait = waits[:1]
        for w in waits[1:]:
            n2 = nc.sync.nop()
            if n2.ins.sync_info is None:
                n2.ins.sync_info = mybir.SyncInfo(on_wait=[w], on_update=[])
            else:
                n2.ins.sync_info.on_wait = [w]
        nc.sync.drain()
        nc.all_engine_barrier()
        popped = nc._tile_sem_poison_stack.pop()
        assert popped is self._sem_poison
        nc.clear_and_free_semaphores(list(self.sems.allocated().values()))
        nc.all_engine_barrier()


def _split_multi_waits(nc):
    for fn in nc.m.functions:
        for blk in fn.blocks:
            new_list = []
            for inst in blk.instructions:
                si = inst.sync_info
                waits = list(si.on_wait) if si and si.on_wait else []
                if len(waits) > 1:
                    for j, w in enumerate(waits[:-1]):
                        nop = mybir.InstNoOp(
                            name=f"{inst.name}-ws{j}",
                            engine=inst.engine,
                            ins=[], outs=[],
                            sync_info=mybir.SyncInfo(on_wait=[w], on_update=[]),
                        )
                        nc.register_instruction(nop, overwrite=True)
                        new_list.append(nop)
                    si.on_wait = waits[-1:]
                new_list.append(inst)
            blk.instructions[:] = new_list


def _dims(ap, dims):
    """Same tensor+offset as `ap`, explicit [stride(elem), nelem] dims."""
    return bass.AP(ap.tensor, ap.offset, dims)


def _build_program(meta):
    nlo, nhi, piece_info, NCH, FCH = meta
    W = NCH * 8
    MAXTC = max(nlo[t] + nhi[t] for t in range(NT))
    nc = bass.Bass("TRN2", target_bir_lowering=False, debug=False, num_devices=NC,
                   num_swdge_queues=4)

    x_sh = nc.dram_tensor("x_sh", [NSH, D], F32, kind="ExternalInput")
    degsh_in = nc.dram_tensor("degsh", [P, NT], F32, kind="ExternalInput")
    idx_in = nc.dram_tensor("idx_e", [P, W], I16, kind="ExternalInput")
    rowloc_in = nc.dram_tensor("rowloc", [P, NCH], F16, kind="ExternalInput")
    degcol_in = nc.dram_tensor("degcol", [P, NCH], F32, kind="ExternalInput")
    fidx_in = nc.dram_tensor("fidx", [P, FCH * 8], I16, kind="ExternalInput")
    iota_in = nc.dram_tensor("iota", [P, P], F16, kind="ExternalInput")

    out_f = nc.dram_tensor("out_f", [FCH * P, 576], F16, kind="ExternalOutput")

    x_bounce = nc.dram_tensor("x_bounce", [NSH, D], F32)
    x_full = nc.dram_tensor("x_full", [NPAD, D], F32, addr_space="Shared")
    y1b = nc.dram_tensor("y1b", [NSH, 256], F16)
    y1full = nc.dram_tensor("y1full", [NPAD, 256], F16, addr_space="Shared")
    ftab = nc.dram_tensor("ftab", [NSH, FT], F16)

    eq = mybir.AluOpType.is_equal
    mult = mybir.AluOpType.mult
    sub = mybir.AluOpType.subtract
    COPY = mybir.ActivationFunctionType.Copy
    SQRT = mybir.ActivationFunctionType.Sqrt

    # one gpsimd register per distinct gather length (gathers are split into
    # <=GMAX-chunk instructions: the SWDGE descriptor ring holds only ~64
    # descriptors per engine, i.e. ~1024 gathered rows per instruction)
    nidx_vals = sorted({P * n for n in range(1, GMAX + 1)} | {P * FPC})
    with _TC(nc) as tc, nc.allow_low_precision(
            reason="fp16 matmul operands and fp16 output; PSUM accumulates f32"), \
            __import__("contextlib").ExitStack() as _regs:
        nidx_reg = {v: _regs.enter_context(nc.gpsimd.register(f"nidx{v}"))
                    for v in nidx_vals}
        for v, r in nidx_reg.items():
            nc.gpsimd.reg_mov(r, v)
        with (
            tc.tile_pool(name="const", bufs=1) as cp,
            tc.tile_pool(name="v1", bufs=3) as v1p,
            tc.tile_pool(name="v2", bufs=3) as v2p,
            tc.tile_pool(name="v3", bufs=3) as v3p,
            tc.tile_pool(name="s", bufs=3) as sp_,
            tc.tile_pool(name="ev", bufs=3) as evp,
            tc.tile_pool(name="x0", bufs=3) as x0p,
            tc.tile_pool(name="g", bufs=2) as gp,
            tc.tile_pool(name="st", bufs=2) as stp,
            tc.tile_pool(name="psum", bufs=4, space="PSUM") as pp,
        ):
            nc.gpsimd.load_library(library_config.mlp)

            iota_t = cp.tile([P, P], F16)
            nc.sync.dma_start(out=iota_t[:], in_=iota_in[:])
            idx_t = cp.tile([P, W], I16)
            nc.sync.dma_start(out=idx_t[:], in_=idx_in[:])
            rowloc_t = cp.tile([P, NCH], F16)
            nc.sync.dma_start(out=rowloc_t[:], in_=rowloc_in[:])
            degcol_t = cp.tile([P, NCH], F32)
            nc.sync.dma_start(out=degcol_t[:], in_=degcol_in[:])
            degsh_t = cp.tile([P, NT], F32)
            nc.sync.dma_start(out=degsh_t[:], in_=degsh_in[:])
            fidx_t = cp.tile([P, FCH * 8], I16)
            nc.sync.dma_start(out=fidx_t[:], in_=fidx_in[:])

            # zero the pad columns of y1b (fp8) and ftab (f16) once
            zpad8 = cp.tile([P, NT, 64], F16)
            nc.vector.memset(zpad8[:], 0.0)
            y1pad = bass.AP(y1b, 192, [[256, P], [256 * P, NT], [1, 64]])
            nc.sync.dma_start(out=y1pad, in_=zpad8[:])
            zpadh = cp.tile([P, NT, 64], F16)
            nc.vector.memset(zpadh[:], 0.0)
            ftpad = bass.AP(ftab, 576, [[FT, P], [FT * P, NT], [1, 64]])
            nc.sync.dma_start(out=ftpad, in_=zpadh[:])

            # per-edge col scales: rq_all[:,0,:]=rsqrt(deg_col), [:,1,:]=sqrt
            rq_all = cp.tile([P, 2, NCH], F16)
            q32_all = cp.tile([P, NCH], F32)
            nc.scalar.activation(q32_all[:], degcol_t[:], SQRT)
            nc.vector.tensor_copy(out=rq_all[:, 1, :], in_=q32_all[:])
            nc.vector.reciprocal(rq_all[:, 0, :], q32_all[:])
            # per-row scales
            degrev_all = cp.tile([P, NT], F32)
            nc.vector.reciprocal(degrev_all[:], degsh_t[:])
            rq0_all = cp.tile([P, 2, NT], F32)
            nc.scalar.activation(rq0_all[:, 1, :], degsh_t[:], SQRT)
            nc.vector.reciprocal(rq0_all[:, 0, :], rq0_all[:, 1, :])

            # materialize the full x table on-device (bounce: collectives
            # cannot read IO tensors directly)
            nc.sync.dma_start(out=x_bounce[:], in_=x_sh[:])
            nc.gpsimd.collective_compute(
                "AllGather", mybir.AluOpType.bypass,
                replica_groups=[list(range(NC))],
                ins=[x_bounce[:]], outs=[x_full[:]],
            )

            def build_s(s_t, a, b, o):
                n = b - a
                rl = rowloc_t[:, a:b]
                rl_b = rl.to_broadcast([P, n, P])
                io = iota_t[:]
                io_b = _dims(io, [io.ap[0], [0, n], io.ap[1]])
                nc.vector.tensor_tensor(out=s_t[:, o:o + n, :], in0=rl_b,
                                        in1=io_b, op=eq)

            qctr = [0]

            def gather(v_tile, voff, table_ap, c0, nchunks):
                # split into <=GMAX-chunk dma_gathers (SWDGE ring limit),
                # round-robin across the 4 SWDGE queues so descriptor
                # generation overlaps ring drain
                done = 0
                while done < nchunks:
                    n = min(GMAX, nchunks - done)
                    a = c0 + done
                    nc.gpsimd.dma_gather(
                        out_ap=v_tile[:, voff + done:voff + done + n, :],
                        in_ap=table_ap,
                        idxs_ap=idx_t[:, 8 * a: 8 * (a + n)],
                        num_idxs=P * n, num_idxs_reg=nidx_reg[P * n],
                        elem_size=table_ap.shape[-1],
                        queue_num=qctr[0] % 4)
                    qctr[0] += 1
                    done += n

            # ================= hop 1 =================
            for (c0, nlo_p, nhi_p, t0, t1) in piece_info:
                v1 = v1p.tile([P, CAP, D], F32, tag="v1")
                if nlo_p:
                    gather(v1, 0, x_full[:], c0, nlo_p)
                if nhi_p:
                    gather(v1, nlo_p, x_full[SPLIT:, :], c0 + nlo_p, nhi_p)
                lo_off, hi_off = 0, nlo_p
                for t in range(t0, t1):
                    ntl, nth = nlo[t], nhi[t]
                    ntc = ntl + nth
                    a_lo, a_hi = c0 + lo_off, c0 + hi_off
                    s_t = sp_.tile([P, MAXTC, P], F16, tag="s")
                    if ntl:
                        build_s(s_t, a_lo, a_lo + ntl, 0)
                    if nth:
                        build_s(s_t, a_hi, a_hi + nth, ntl)
                    v3 = v3p.tile([P, MAXTC, 192], F16, tag="v3")
                    ap0 = v3[:].ap[0]

                    def expand(voff, n, o, a):
                        b0 = v3[:, o:o + n, 0:D]
                        nc.scalar.activation(b0, v1[:, voff:voff + n, :], COPY)
                        b12 = _dims(v3[:, o:o + n, D:3 * D],
                                    [ap0, [192, n], [D, 2], [1, D]])
                        v16b = _dims(v3[:, o:o + n, 0:D],
                                     [ap0, [192, n], [0, 2], [1, D]])
                        rqb = _dims(rq_all[:, :, a:a + n],
                                    [rq_all[:].ap[0], [1, n], [NCH, 2], [0, D]])
                        nc.vector.tensor_tensor(out=b12, in0=v16b, in1=rqb,
                                                op=mult)
                    if ntl:
                        expand(lo_off, ntl, 0, a_lo)
                    if nth:
                        expand(hi_off, nth, ntl, a_hi)
                    ps = pp.tile([P, 192], F32, space="PSUM", tag="ps")
                    for j in range(ntc):
                        nc.tensor.matmul(
                            out=ps[:], lhsT=s_t[:, j, :], rhs=v3[:, j, :],
                            start=(j == 0), stop=(j == ntc - 1))
                    y1t = evp.tile([P, 192], F16, tag="y1")
                    nc.scalar.activation(y1t[:], ps[:], COPY,
                                         scale=degrev_all[:, t:t + 1])
                    y1dst = _dims(y1b[t * P:(t + 1) * P, 0:192],
                                  [[256, P], [1, 192]])
                    nc.sync.dma_start(out=y1dst, in_=y1t[:])
                    fdst = _dims(ftab[t * P:(t + 1) * P, 192:384],
                                 [[FT, P], [1, 192]])
                    nc.sync.dma_start(out=fdst, in_=y1t[:])
                    lo_off += ntl
                    hi_off += nth

            # ================= AllGather =================
            nc.gpsimd.collective_compute(
                "AllGather", mybir.AluOpType.bypass,
                replica_groups=[list(range(NC))],
                ins=[y1b[:]], outs=[y1full[:]],
            )

            # ================= hop 2 =================
            for (c0, nlo_p, nhi_p, t0, t1) in piece_info:
                v2 = v2p.tile([P, CAP, 256], F16, tag="v2")
                if nlo_p:
                    gather(v2, 0, y1full[:], c0, nlo_p)
                if nhi_p:
                    gather(v2, nlo_p, y1full[SPLIT:, :], c0 + nlo_p, nhi_p)
                lo_off, hi_off = 0, nlo_p
                for t in range(t0, t1):
                    ntl, nth = nlo[t], nhi[t]
                    ntc = ntl + nth
                    a_lo, a_hi = c0 + lo_off, c0 + hi_off
                    s_t = sp_.tile([P, MAXTC, P], F16, tag="s")
                    if ntl:
                        build_s(s_t, a_lo, a_lo + ntl, 0)
                    if nth:
                        build_s(s_t, a_hi, a_hi + nth, ntl)
                    ps = pp.tile([P, 192], F32, space="PSUM", tag="ps")
                    j = 0
                    for (voff, n) in ((lo_off, ntl), (hi_off, nth)):
                        for k in range(n):
                            nc.tensor.matmul(
                                out=ps[:], lhsT=s_t[:, j, :],
                                rhs=v2[:, voff + k, 0:192],
                                start=(j == 0), stop=(j == ntc - 1))
                            j += 1
                    # xs0 = [x, x*rsqrt(deg), x*sqrt(deg)] in f16
                    x_t = x0p.tile([P, D], F32, tag="xt")
                    nc.sync.dma_start(out=x_t[:],
                                      in_=x_sh[t * P:(t + 1) * P, :])
                    xs0 = x0p.tile([P, 192], F16, tag="xs0")
                    nc.scalar.activation(xs0[:, 0:D], x_t[:], COPY)
                    xb = _dims(x_t[:], [x_t[:].ap[0], [0, 2], [1, D]])
                    rq0b = _dims(rq0_all[:, :, t:t + 1],
                                 [rq0_all[:].ap[0], [NT, 2], [0, D]])
                    b12 = _dims(xs0[:, D:3 * D],
                                [xs0[:].ap[0], [D, 2], [1, D]])
                    nc.vector.tensor_tensor(out=b12, in0=xb, in1=rq0b, op=mult)
                    tmp = evp.tile([P, 192], F16, tag="tmp2")
                    nc.scalar.activation(tmp[:], ps[:], COPY,
                                         scale=degrev_all[:, t:t + 1])
                    xs2 = evp.tile([P, 192], F16, tag="xs2")
                    nc.vector.tensor_tensor(out=xs2[:], in0=tmp[:],
                                            in1=xs0[:], op=sub)
                    f0 = _dims(ftab[t * P:(t + 1) * P, 0:192],
                               [[FT, P], [1, 192]])
                    nc.sync.dma_start(out=f0, in_=xs0[:])
                    f2 = _dims(ftab[t * P:(t + 1) * P, 384:576],
                               [[FT, P], [1, 192]])
                    nc.sync.dma_start(out=f2, in_=xs2[:])
                    lo_off += ntl
                    hi_off += nth

            # ================= final gather + transpose =================
            for fp_i in range(FCH // FPC):
                g = gp.tile([P, FPC, FT], F16, tag="g")
                nc.gpsimd.dma_gather(
                    out_ap=g[:], in_ap=ftab[:],
                    idxs_ap=fidx_t[:, fp_i * FPC * 8:(fp_i + 1) * FPC * 8],
                    num_idxs=P * FPC, num_idxs_reg=nidx_reg[P * FPC],
                    elem_size=FT, queue_num=qctr[0] % 4)
                qctr[0] += 1
                stage = stp.tile([P, FPC, 576], F16, tag="stage")
                for k in range(9):
                    src = g[:, :, k * D:(k + 1) * D]
                    dst = _dims(stage[:, :, k:k + 1],
                                [stage[:].ap[0], [576, FPC], [9, D]])
                    if k % 2 == 0:
                        nc.vector.tensor_copy(out=dst, in_=src)
                    else:
                        nc.scalar.activation(dst, src, COPY)
                obase = out_f[fp_i * FPC * P:(fp_i + 1) * FPC * P, :]
                orows = _dims(obase, [[576, P], [P * 576, FPC], [1, 576]])
                nc.sync.dma_start(out=orows, in_=stage[:])

    lower_extended_insts(nc)
    _split_multi_waits(nc)
    return nc


def _plan(x, deg, adj_row, adj_col, edge):
    """Host-side sharding: pure index bookkeeping + input reordering."""
    x = np.asarray(x, np.float32)
    deg = np.asarray(deg, np.float32).reshape(-1)
    adj_row = np.asarray(adj_row, np.int64)
    adj_col = np.asarray(adj_col, np.int64)
    ep = np.asarray(edge, np.int64).reshape(-1)

    seg = np.searchsorted(adj_row, np.arange(NC + 1) * NSH)
    percore = []
    cnt = np.zeros((NC, NT, 2), np.int64)
    for c in range(NC):
        rl = adj_row[seg[c]:seg[c + 1]] - c * NSH
        co = adj_col[seg[c]:seg[c + 1]]
        t = rl >> 7
        s = (co >= SPLIT).astype(np.int64)
        key = t * 2 + s
        order = np.argsort(key, kind="stable")
        rl, co, key = rl[order], co[order], key[order]
        cnt[c] = np.bincount(key, minlength=NT * 2).reshape(NT, 2)
        percore.append((rl, co, key))
    nch = -(-cnt.max(axis=0) // P)          # [NT, 2] chunks per (tile, side)
    assert (nch.sum(axis=1) > 0).all()

    # piece partition: greedy tile grouping with sum(chunks) <= CAP
    pieces = []
    t0 = 0
    while t0 < NT:
        t1 = t0 + 1
        while t1 < NT and nch[t0:t1 + 1].sum() <= CAP:
            t1 += 1
        pieces.append((t0, t1))
        t0 = t1
    chunk_base = np.zeros((NT, 2), np.int64)
    pos = 0
    piece_info = []
    for (t0, t1) in pieces:
        c0 = pos
        for t in range(t0, t1):
            chunk_base[t, 0] = pos
            pos += nch[t, 0]
        nlo_p = pos - c0
        for t in range(t0, t1):
            chunk_base[t, 1] = pos
            pos += nch[t, 1]
        nhi_p = pos - c0 - nlo_p
        piece_info.append((int(c0), int(nlo_p), int(nhi_p), int(t0), int(t1)))
    NCH = int(pos)

    mines = []
    for c in range(NC):
        r0 = c * NSH
        mines.append(np.nonzero((ep >= r0) & (ep < r0 + NSH))[0])
    FCH = -(-max(len(m) for m in mines) // P)
    FCH = -(-FCH // FPC) * FPC

    meta = (tuple(int(v) for v in nch[:, 0]),
            tuple(int(v) for v in nch[:, 1]),
            tuple(piece_info), NCH, int(FCH))

    iota_np = np.tile(np.arange(P, dtype=np.float16), (P, 1))
    in_maps, positions = [], []
    for c in range(NC):
        rl, co, key = percore[c]
        gcnt = np.bincount(key, minlength=NT * 2)
        gstart = np.zeros(NT * 2 + 1, np.int64)
        np.cumsum(gcnt, out=gstart[1:])
        idx_list = np.zeros(NCH * P, np.int16)
        rowloc = np.full(NCH * P, -1.0, np.float16)
        degcol = np.ones(NCH * P, np.float32)
        for t in range(NT):
            for s in range(2):
                g0, g1 = gstart[t * 2 + s], gstart[t * 2 + s + 1]
                n = g1 - g0
                if n == 0:
                    continue
                j0 = chunk_base[t, s] * P
                idx_list[j0:j0 + n] = (co[g0:g1] - s * SPLIT).astype(np.int16)
                rowloc[j0:j0 + n] = (rl[g0:g1] - (t << 7)).astype(np.float16)
                degcol[j0:j0 + n] = deg[co[g0:g1]]
        idx16 = np.tile(np.ascontiguousarray(idx_list.reshape(-1, 16).T),
                        (8, 1))
        rowloc_a = np.ascontiguousarray(rowloc.reshape(NCH, P).T)
        degcol_a = np.ascontiguousarray(degcol.reshape(NCH, P).T)

        r0 = c * NSH
        real = min(NSH, max(0, N - r0))
        dloc = np.ones(NSH, np.float32)
        dloc[:real] = deg[r0:r0 + real]
        degsh = np.ascontiguousarray(dloc.reshape(NT, P).T)
        x_shard = np.zeros((NSH, D), np.float32)
        x_shard[:real] = x[r0:r0 + real]

        mine = mines[c]
        fidx_l = np.zeros(FCH * P, np.int16)
        fidx_l[:len(mine)] = (ep[mine] - r0).astype(np.int16)
        fidx16 = np.tile(np.ascontiguousarray(fidx_l.reshape(-1, 16).T),
                         (8, 1))
        positions.append(mine)
        in_maps.append({
            "x_sh": x_shard,
            "degsh": degsh,
            "idx_e": idx16,
            "rowloc": rowloc_a,
            "degcol": degcol_a,
            "fidx": fidx16,
            "iota": iota_np,
        })
    return in_maps, positions, meta


def _assemble(results, positions):
    out = np.zeros((2 * EQ, 576), np.float32)
    for c in range(NC):
        rows = results[c]["out_f"]
        n_c = len(positions[c])
        out[positions[c]] = rows[:n_c]
    return out.reshape(2, EQ, D, 9)


def kernel(x, deg, adj_row, adj_col, edge):
    import time
    t0 = time.time()
    in_maps, positions, meta = _plan(x, deg, adj_row, adj_col, edge)
    print(f"[kernel] host plan: {time.time()-t0:.1f}s", flush=True)
    if meta not in _prog_cache:
        t0 = time.time()
        _prog_cache[meta] = _build_program(meta)
        print(f"[kernel] program build: {time.time()-t0:.1f}s", flush=True)
    nc = _prog_cache[meta]
    t0 = time.time()
    res = run_bass_kernel_spmd(nc, in_maps, list(range(NC)))
    print(f"[kernel] compile+run: {time.time()-t0:.1f}s", flush=True)
    return _assemble(res.results, positions)
